# revision 9
# baseline (speedup 1.0000x reference)
"""Correlation network kernel for Trainium2.

corr[b,i,j,k,l] = sum_c A[b,i,j,c] * B[b,k,l,c]

Per batch b this is  A_b (2304x64) @ B_b^T (64x2304) -> 2304x2304.
Sharding: data-parallel over batch B=8 across the 8 NeuronCores; each core
computes one full 2304x2304 correlation matrix (21.2 MB fp32 out), so the
kernel is output-write bound (~358 GB/s HBM per core => ~60 us floor).

Device-side plan (per core):
  - Inputs arrive host-prepped: transposed to [C, HW] layout and split into
    bf16 hi/lo pairs (A = A_hi + A_lo captures ~17 mantissa bits, giving
    ~5e-6 relative output error vs the fp32 reference).
    Host prep removes all on-device transposes and keeps full precision
    without fp32 matmuls (8 cycles/row effective) or fp32r (~1e-4 error).
  - K=C=64 uses only half the 128-row PE array, so m-tiles are packed in
    pairs: even m-tiles occupy array rows 0-63, odd m-tiles rows 64-127
    (tile_position auto-derived from SBUF base partition). The two groups'
    matmuls run concurrently and each group's LDWEIGHTS overlaps the other
    group's matmuls. B^T operands are duplicated into both partition halves
    so the moving operand streams into the matching array rows.
  - Per (m-pair, n-tile): 6 bf16 matmuls (2 groups x {hi*hi, hi*lo, lo*hi})
    accumulating into two PSUM banks, then PSUM->SBUF copies balanced
    across DVE and ACT, then one 1.18 MB contiguous DMA per m row-block.
"""

import numpy as np
import ml_dtypes

import concourse.bacc as bacc
import concourse.mybir as mybir
import concourse.tile as tile
from concourse.bass_interp import get_hw_module
from concourse.bass_utils import run_bass_kernel_spmd

B, H, W, C = 8, 48, 48, 64
HW = H * W  # 2304
P = 128
M_TILES = HW // P  # 18
M_PAIRS = M_TILES // 2  # 9
N_TILE = 512
FP32 = mybir.dt.float32
BF16 = mybir.dt.bfloat16
BF16_NP = ml_dtypes.bfloat16

N_SPLITS = []
_n0 = 0
while _n0 < HW:
    N_SPLITS.append((_n0, min(N_TILE, HW - _n0)))
    _n0 += N_TILE


def _corr_body(tc, out, a_hi, a_lo, b_hi, b_lo):
    nc = tc.nc
    with (
        tc.tile_pool(name="ops", bufs=1) as op_pool,
        tc.tile_pool(name="ps", bufs=6, space="PSUM") as ps_pool,
        tc.tile_pool(name="outs", bufs=4) as out_pool,
    ):
        # lhsT operands: [128, 1152]; rows 0:64 = even m-tiles, 64:128 = odd
        ath = op_pool.tile([P, HW // 2], BF16)
        atl = op_pool.tile([P, HW // 2], BF16)
        # rhs operands: [128, 2304]; rows 64:128 duplicate rows 0:64
        bth = op_pool.tile([P, HW], BF16)
        btl = op_pool.tile([P, HW], BF16)
        # Inputs go through SWDGE (gpsimd) so they never queue behind the
        # HWDGE output rings; ordered so the first matmuls' operands land
        # first (terms are hh, hl, lh).
        for t, src in ((ath, a_hi), (bth, b_hi), (btl, b_lo), (atl, a_lo)):
            nc.gpsimd.dma_start(out=t[:, :], in_=src[:, :])

        for p in range(M_PAIRS):
            ot_e = out_pool.tile([P, HW], FP32, tag="ot")
            ot_o = out_pool.tile([P, HW], FP32, tag="ot")
            col = slice(p * P, (p + 1) * P)
            for ni, (n0, nsz) in enumerate(N_SPLITS):
                ps_e = ps_pool.tile([P, N_TILE], FP32, tag="ps")
                ps_o = ps_pool.tile([P, N_TILE], FP32, tag="ps")
                terms = ((ath, bth), (ath, btl), (atl, bth))
                for k, (at, bt) in enumerate(terms):
                    st, sp = k == 0, k == len(terms) - 1
                    nc.tensor.matmul(
                        ps_e[:, :nsz],
                        at[0:64, col],
                        bt[0:64, n0 : n0 + nsz],
                        start=st,
                        stop=sp,
                    )
                    nc.tensor.matmul(
                        ps_o[:, :nsz],
                        at[64:128, col],
                        bt[64:128, n0 : n0 + nsz],
                        start=st,
                        stop=sp,
                    )
                # balance the PSUM->SBUF copies across DVE and ACT
                if ni % 2 == 0:
                    nc.vector.tensor_copy(ot_e[:, n0 : n0 + nsz], ps_e[:, :nsz])
                    nc.scalar.copy(ot_o[:, n0 : n0 + nsz], ps_o[:, :nsz])
                else:
                    nc.scalar.copy(ot_e[:, n0 : n0 + nsz], ps_e[:, :nsz])
                    nc.vector.tensor_copy(ot_o[:, n0 : n0 + nsz], ps_o[:, :nsz])
            # alternate the two HWDGE rings (SP via nc.sync, ACT via
            # nc.scalar) so row-block stores drain in parallel
            m_e, m_o = 2 * p, 2 * p + 1
            nc.sync.dma_start(out=out[m_e * P : (m_e + 1) * P, :], in_=ot_e[:, :])
            nc.scalar.dma_start(out=out[m_o * P : (m_o + 1) * P, :], in_=ot_o[:, :])


_NC_CACHE = None


def _build():
    global _NC_CACHE
    if _NC_CACHE is None:
        nc = bacc.Bacc(
            "TRN2",
            target_bir_lowering=False,
            debug=False,
            enable_asserts=False,
        )
        a_hi = nc.dram_tensor("a_hi", [P, HW // 2], BF16, kind="ExternalInput").ap()
        a_lo = nc.dram_tensor("a_lo", [P, HW // 2], BF16, kind="ExternalInput").ap()
        b_hi = nc.dram_tensor("b_hi", [P, HW], BF16, kind="ExternalInput").ap()
        b_lo = nc.dram_tensor("b_lo", [P, HW], BF16, kind="ExternalInput").ap()
        out = nc.dram_tensor("out", [HW, HW], FP32, kind="ExternalOutput").ap()
        with tile.TileContext(nc) as tc:
            _corr_body(tc, out, a_hi, a_lo, b_hi, b_lo)
        nc.compile()
        nc.m = get_hw_module(nc.m)
        _NC_CACHE = nc
    return _NC_CACHE


def _split_hi_lo(x):
    """x: [HW, C] fp32 -> (hi, lo) bf16 with x ~= hi + lo."""
    hi = x.astype(BF16_NP)
    lo = (x - hi.astype(np.float32)).astype(BF16_NP)
    return hi, lo


def _pack_lhs(xT):
    """[C, HW] -> [128, HW/2]: rows 0:64 even m-tiles, rows 64:128 odd."""
    t = xT.reshape(C, M_PAIRS, 2, P)  # [c, pair, eo, j]
    return np.ascontiguousarray(t.transpose(2, 0, 1, 3).reshape(2 * C, M_PAIRS * P))


def _pack_rhs(xT):
    """[C, HW] -> [128, HW]: duplicate into both partition halves."""
    return np.ascontiguousarray(np.concatenate([xT, xT], axis=0))


def _prep_inputs(feature_A, feature_B):
    in_maps = []
    for i in range(B):
        A2 = np.ascontiguousarray(feature_A[i].reshape(HW, C), dtype=np.float32)
        B2 = np.ascontiguousarray(feature_B[i].reshape(HW, C), dtype=np.float32)
        ah, al = _split_hi_lo(A2)
        bh, bl = _split_hi_lo(B2)
        in_maps.append(
            {
                "a_hi": _pack_lhs(np.ascontiguousarray(ah.T)),
                "a_lo": _pack_lhs(np.ascontiguousarray(al.T)),
                "b_hi": _pack_rhs(np.ascontiguousarray(bh.T)),
                "b_lo": _pack_rhs(np.ascontiguousarray(bl.T)),
            }
        )
    return in_maps


def _run(feature_A, feature_B, trace=False, **kwargs):
    feature_A = np.asarray(feature_A, dtype=np.float32)
    feature_B = np.asarray(feature_B, dtype=np.float32)
    assert feature_A.shape == (B, H, W, C), feature_A.shape
    assert feature_B.shape == (B, H, W, C), feature_B.shape

    nc = _build()
    in_maps = _prep_inputs(feature_A, feature_B)
    res = run_bass_kernel_spmd(nc, in_maps, list(range(B)), trace=trace, **kwargs)
    out = np.stack([res.results[i]["out"] for i in range(B)], axis=0)
    return out.reshape(B, H, W, H, W), res


def kernel(feature_A, feature_B):
    out, _ = _run(feature_A, feature_B)
    return out


# revision 12
# speedup vs baseline: 1.1365x; 1.1365x over previous
"""Correlation network kernel for Trainium2.

corr[b,i,j,k,l] = sum_c A[b,i,j,c] * B[b,k,l,c]

Per batch b this is  A_b (2304x64) @ B_b^T (64x2304) -> 2304x2304.
Sharding: data-parallel over batch B=8 across the 8 NeuronCores; each core
computes one full 2304x2304 correlation matrix (21.2 MB fp32 out), so the
kernel is output-write bound (~358 GB/s HBM per core => ~60 us floor).

Device-side plan (per core):
  - Inputs arrive host-prepped: transposed to [C, HW] layout and split into
    bf16 hi/lo pairs (A = A_hi + A_lo captures ~17 mantissa bits, giving
    ~5e-6 relative output error vs the fp32 reference).
    Host prep removes all on-device transposes and keeps full precision
    without fp32 matmuls (8 cycles/row effective) or fp32r (~1e-4 error).
  - K=C=64 uses only half the 128-row PE array, so m-tiles are packed in
    pairs: even m-tiles occupy array rows 0-63, odd m-tiles rows 64-127
    (tile_position auto-derived from SBUF base partition). The two groups'
    matmuls run concurrently and each group's LDWEIGHTS overlaps the other
    group's matmuls. B^T operands are duplicated into both partition halves
    so the moving operand streams into the matching array rows.
  - Per (m-pair, n-tile): 6 bf16 matmuls (2 groups x {hi*hi, hi*lo, lo*hi})
    accumulating into two PSUM banks, then PSUM->SBUF copies balanced
    across DVE and ACT, then one 1.18 MB contiguous DMA per m row-block.
"""

import numpy as np
import ml_dtypes

import concourse.bacc as bacc
import concourse.mybir as mybir
import concourse.tile as tile
from concourse.bass_interp import get_hw_module
from concourse.bass_utils import run_bass_kernel_spmd

B, H, W, C = 8, 48, 48, 64
HW = H * W  # 2304
P = 128
M_TILES = HW // P  # 18
M_PAIRS = M_TILES // 2  # 9
N_TILE = 512
FP32 = mybir.dt.float32
BF16 = mybir.dt.bfloat16
BF16_NP = ml_dtypes.bfloat16

N_SPLITS = []
_n0 = 0
while _n0 < HW:
    N_SPLITS.append((_n0, min(N_TILE, HW - _n0)))
    _n0 += N_TILE


def _corr_body(tc, out, a_hi, a_lo, b_hi, b_lo):
    nc = tc.nc
    with (
        tc.tile_pool(name="ops", bufs=1) as op_pool,
        tc.tile_pool(name="ps", bufs=8, space="PSUM") as ps_pool,
        tc.tile_pool(name="outs", bufs=4) as out_pool,
    ):
        # lhsT operands: [128, 1152]; rows 0:64 = even m-tiles, 64:128 = odd
        ath = op_pool.tile([P, HW // 2], BF16)
        atl = op_pool.tile([P, HW // 2], BF16)
        # rhs operands: [128, 2304]; rows 64:128 duplicate rows 0:64
        bth = op_pool.tile([P, HW], BF16)
        btl = op_pool.tile([P, HW], BF16)
        # Inputs go through SWDGE (gpsimd) so they never queue behind the
        # HWDGE output ring. Split each load so the first m-pair's operand
        # chunks land first (terms are hh, hl, lh), letting matmuls start
        # ~4 us earlier; the remainders stream in behind them.
        first = []
        rest = []
        for t, src, c in (
            (ath, a_hi, P),
            (bth, b_hi, N_TILE),
            (btl, b_lo, N_TILE),
            (atl, a_lo, P),
        ):
            first.append((t[:, :c], src[:, :c]))
            rest.append((t[:, c:], src[:, c:]))
        for dst, src in first + rest:
            nc.gpsimd.dma_start(out=dst, in_=src)

        for p in range(M_PAIRS):
            ot_e = out_pool.tile([P, HW], FP32, tag="ot")
            ot_o = out_pool.tile([P, HW], FP32, tag="ot")
            col = slice(p * P, (p + 1) * P)
            for ni, (n0, nsz) in enumerate(N_SPLITS):
                ps_e = ps_pool.tile([P, N_TILE], FP32, tag="ps")
                ps_o = ps_pool.tile([P, N_TILE], FP32, tag="ps")
                terms = ((ath, bth), (ath, btl), (atl, bth))
                for k, (at, bt) in enumerate(terms):
                    st, sp = k == 0, k == len(terms) - 1
                    nc.tensor.matmul(
                        ps_e[:, :nsz],
                        at[0:64, col],
                        bt[0:64, n0 : n0 + nsz],
                        start=st,
                        stop=sp,
                    )
                    nc.tensor.matmul(
                        ps_o[:, :nsz],
                        at[64:128, col],
                        bt[64:128, n0 : n0 + nsz],
                        start=st,
                        stop=sp,
                    )
                # balance the PSUM->SBUF copies across DVE and ACT
                if ni % 2 == 0:
                    nc.vector.tensor_copy(ot_e[:, n0 : n0 + nsz], ps_e[:, :nsz])
                    nc.scalar.copy(ot_o[:, n0 : n0 + nsz], ps_o[:, :nsz])
                else:
                    nc.scalar.copy(ot_e[:, n0 : n0 + nsz], ps_e[:, :nsz])
                    nc.vector.tensor_copy(ot_o[:, n0 : n0 + nsz], ps_o[:, :nsz])
            # All row-block stores go on the SP HWDGE ring (keeping DMA
            # issue off the scalar engine, whose copies gate PSUM reuse) --
            # except the final block, which rides the ACT ring so the last
            # two stores drain in parallel instead of back-to-back.
            m_e, m_o = 2 * p, 2 * p + 1
            nc.sync.dma_start(out=out[m_e * P : (m_e + 1) * P, :], in_=ot_e[:, :])
            last = p == M_PAIRS - 1
            eng = nc.scalar if last else nc.sync
            eng.dma_start(out=out[m_o * P : (m_o + 1) * P, :], in_=ot_o[:, :])


_NC_CACHE = None


def _build():
    global _NC_CACHE
    if _NC_CACHE is None:
        nc = bacc.Bacc(
            "TRN2",
            target_bir_lowering=False,
            debug=False,
            enable_asserts=False,
        )
        a_hi = nc.dram_tensor("a_hi", [P, HW // 2], BF16, kind="ExternalInput").ap()
        a_lo = nc.dram_tensor("a_lo", [P, HW // 2], BF16, kind="ExternalInput").ap()
        b_hi = nc.dram_tensor("b_hi", [P, HW], BF16, kind="ExternalInput").ap()
        b_lo = nc.dram_tensor("b_lo", [P, HW], BF16, kind="ExternalInput").ap()
        out = nc.dram_tensor("out", [HW, HW], FP32, kind="ExternalOutput").ap()
        with tile.TileContext(nc) as tc:
            _corr_body(tc, out, a_hi, a_lo, b_hi, b_lo)
        nc.compile()
        nc.m = get_hw_module(nc.m)
        _NC_CACHE = nc
    return _NC_CACHE


def _split_hi_lo(x):
    """x: [HW, C] fp32 -> (hi, lo) bf16 with x ~= hi + lo."""
    hi = x.astype(BF16_NP)
    lo = (x - hi.astype(np.float32)).astype(BF16_NP)
    return hi, lo


def _pack_lhs(xT):
    """[C, HW] -> [128, HW/2]: rows 0:64 even m-tiles, rows 64:128 odd."""
    t = xT.reshape(C, M_PAIRS, 2, P)  # [c, pair, eo, j]
    return np.ascontiguousarray(t.transpose(2, 0, 1, 3).reshape(2 * C, M_PAIRS * P))


def _pack_rhs(xT):
    """[C, HW] -> [128, HW]: duplicate into both partition halves."""
    return np.ascontiguousarray(np.concatenate([xT, xT], axis=0))


def _prep_inputs(feature_A, feature_B):
    in_maps = []
    for i in range(B):
        A2 = np.ascontiguousarray(feature_A[i].reshape(HW, C), dtype=np.float32)
        B2 = np.ascontiguousarray(feature_B[i].reshape(HW, C), dtype=np.float32)
        ah, al = _split_hi_lo(A2)
        bh, bl = _split_hi_lo(B2)
        in_maps.append(
            {
                "a_hi": _pack_lhs(np.ascontiguousarray(ah.T)),
                "a_lo": _pack_lhs(np.ascontiguousarray(al.T)),
                "b_hi": _pack_rhs(np.ascontiguousarray(bh.T)),
                "b_lo": _pack_rhs(np.ascontiguousarray(bl.T)),
            }
        )
    return in_maps


def _run(feature_A, feature_B, trace=False, **kwargs):
    feature_A = np.asarray(feature_A, dtype=np.float32)
    feature_B = np.asarray(feature_B, dtype=np.float32)
    assert feature_A.shape == (B, H, W, C), feature_A.shape
    assert feature_B.shape == (B, H, W, C), feature_B.shape

    nc = _build()
    in_maps = _prep_inputs(feature_A, feature_B)
    res = run_bass_kernel_spmd(nc, in_maps, list(range(B)), trace=trace, **kwargs)
    out = np.stack([res.results[i]["out"] for i in range(B)], axis=0)
    return out.reshape(B, H, W, H, W), res


def kernel(feature_A, feature_B):
    out, _ = _run(feature_A, feature_B)
    return out
